# revision 60
# baseline (speedup 1.0000x reference)
"""TRN2 Bass kernel for nn_Block1_43542378447225 (v2, latency-tuned).

Per sample on one NeuronCore (batch=2 -> cores 0/1 do real work):
  conv1 (bias folded into matmul via ones row) -> relu into padded a1p
  -> conv2 (split p-halves; b2 folded via all-ones a1p partition 32)
  -> z2 -> Hopfield #1 in S^T layout -> backward split C = Cz - Cq:
       Cz = Scomb^T((w2b^T z2) . M1W)  [runs during hop1 softmax window]
       Cq = Scomb^T((w2b^T (q~ . m2)) . M1WR)
     where M1WR = M1W * (-64/s) folds the softmax normalization (a per-
     column scale commutes through both partition-contracting matmuls)
     and Cq accumulates into Cz's PSUM tile, so C needs one PSUM->SBUF
     convert -> blocked e_min (single staged cand tile + shifted stack +
     min-reduce) -> eW gather -> mask -> masked patch forward (W1big)
  -> z2_masked -> Hopfield #2 -> out (host divides q2*64/s2).

All SBUF data fp16 (PE 4x faster than fp32; DVE 2x/4x modes need all-
fp16-SBUF operands); PSUM fp32.  The C -> e_min -> mask comparison path
stays bit-exact in fp16: cand/eW matmuls are one-hot gathers, so every
candidate equals an fp16-rounded C entry exactly and the argmin survives
`C16 <= eW`.

Layout: pq = p*8+q (64 conv2 output positions), uv = u*10+v (100
composite window offsets), kc = a*32 + c1 (hidden index; chunk t = conv2
kernel row, a = conv2 kernel col).
"""
import numpy as np

import concourse.bass as bass
import concourse.bacc as bacc
import concourse.mybir as mybir
import concourse.tile as tile
from concourse.tile import add_dep_helper
from concourse.bass_utils import run_bass_kernel_spmd

F32 = mybir.dt.float32
F16 = mybir.dt.float16
AF = mybir.ActivationFunctionType
ALU = mybir.AluOpType

N_CORES = 8
BETA = 0.125  # 1/sqrt(64)

_CACHE = {}


# ---------------------------------------------------------------- host prep
def _build_scomb_w1big(w1):
    w1s = w1.sum(axis=1)
    Scomb = np.zeros((4, 32, 4, 100), np.float32)  # [a, c1, t, uv]
    W1big = np.zeros((100, 3, 4, 4, 32), np.float32)  # [uv, h, t, a, c1]
    for t in range(4):
        for a in range(4):
            for u in range(10):
                ki = u - 2 * t
                if not (0 <= ki < 4):
                    continue
                for v in range(10):
                    kj = v - 2 * a
                    if not (0 <= kj < 4):
                        continue
                    Scomb[a, :, t, u * 10 + v] = w1s[:, ki, kj]
                    W1big[u * 10 + v, :, t, a, :] = w1[:, :, ki, kj].T
    # partition index = a*32+c1 -> merge (a, c1); free = t*100+uv
    return Scomb.reshape(128, 400), W1big.reshape(100, 1536)


def _host_prep(w1, b1, w2, b2, K, Vw):
    # cv1 template [49, 288]: per-sample P1 (cols 0:256, row 48 = ones)
    # filled later; w1f cols 256:288 with b1 folded into row 48.
    main = np.zeros((49, 288), np.float16)
    main[0:48, 256:288] = np.transpose(w1, (2, 3, 1, 0)).reshape(48, 32)
    main[48, 256:288] = b1

    # conv2 weights, every (t, a) block based at partition 0:
    # cv2[c1, (t*4+a)*64 + o] = w2[o, c1, t, a]; b2 folded into partition
    # 32 of block (0,0) -- a1p partition 32 is all-ones on device.
    cv2 = np.zeros((33, 1024), np.float16)
    cv2[0:32, :] = np.transpose(w2, (1, 2, 3, 0)).reshape(32, 1024)
    cv2[32, 0:64] = b2

    # hop [128, 1280]: KT | KV chunks | w2b (2.0 folded, rows 0:64 -- matmul
    # lhsT must share the rhs base partition).  Shipping w2b here (instead
    # of wB) lets the Gz matmuls run inside the hop1 softmax window.
    hop = np.zeros((128, 1280), np.float16)
    hop[0:64, 0:512] = K.T
    # KV chunks [128, 4, 64]: KV[m, e] = (K @ Vw)[m, e] / 64 (the 1/64 keeps
    # the unnormalized q inside fp16 range; -64/s is folded back via M1WR)
    KVh = (K @ Vw).astype(np.float32).reshape(4, 128, 64) / 64.0
    hop[:, 512:768] = np.transpose(KVh, (1, 0, 2)).reshape(128, 256)
    hop[0:64, 768:1280] = 2.0 * np.transpose(w2, (0, 2, 3, 1)).reshape(64, 512)

    Scomb, W1big = _build_scomb_w1big(w1)
    PermF = np.zeros((100, 9, 16), np.float32)
    for k in range(9):
        dp, dq = k // 3 - 1, k % 3 - 1
        for im in range(4):
            u = 4 * dp + im + 3
            if not (0 <= u < 10):
                continue
            for jm in range(4):
                v = 4 * dq + jm + 3
                if not (0 <= v < 10):
                    continue
                PermF[u * 10 + v, k, im * 4 + jm] = 1.0
    CandM = np.zeros((100, 3, 128), np.float32)
    for k in range(9):
        cc, kk = divmod(k, 4)
        CandM[:, cc, kk * 32:kk * 32 + 16] = PermF[:, k, :]
    PermB = np.transpose(PermF, (2, 1, 0)).reshape(16, 900)

    # wB [128, 1784]: Scomb | CandM | PermB | -I (the eW accumulation
    # subtracts C16 so the mask compare fuses into one scalar_tensor_tensor)
    wB = np.zeros((128, 1784), np.float16)
    wB[:, 0:400] = Scomb
    wB[0:100, 400:784] = CandM.reshape(100, 384)
    wB[0:16, 784:1684] = PermB
    wB[0:100, 1684:1784] = -np.eye(100, dtype=np.float16)

    wC = np.zeros((128, 1792), np.float16)
    wC[0:100, 0:1536] = W1big
    wC[:, 1536:1792] = np.transpose(w2, (3, 1, 2, 0)).reshape(128, 256)
    return {"main": main, "cv2": cv2, "hop": hop, "wB": wB, "wC": wC}


def _sample_prep(x_s):
    xp1 = np.pad(x_s, ((0, 0), (1, 1), (1, 1)))
    xp3 = np.pad(x_s, ((0, 0), (3, 3), (3, 3)))
    P1 = np.zeros((4, 4, 3, 16, 16), np.float32)
    for kr in range(4):
        for ks in range(4):
            P1[kr, ks] = xp1[:, kr:kr + 32:2, ks:ks + 32:2][:, :16, :16]
    X = np.zeros((10, 10, 3, 8, 8), np.float32)
    for u in range(10):
        for v in range(10):
            X[u, v] = xp3[:, u:u + 32:4, v:v + 32:4][:, :8, :8]
    P1f = np.ones((49, 256), np.float32)
    P1f[0:48] = P1.reshape(48, 256)
    return (P1f.astype(np.float16), X.reshape(100, 192).astype(np.float16))


# ---------------------------------------------------------------- device build
def _build_nc(debug=False):
    # Route two of the four Bacc-preamble const memsets (all Pool) to DVE
    # so they parallelize and the start barrier clears ~200ns earlier.
    orig_memset = bass.BassGpSimd.memset
    state = {"n": 0}

    def routed_memset(self, ap, constant):
        state["n"] += 1
        if state["n"] in (2, 3, 4):
            return bass.BassVectorEngine.memset(self.bass.vector, ap, constant)
        return orig_memset(self, ap, constant)

    bass.BassGpSimd.memset = routed_memset
    try:
        nc = bacc.Bacc("TRN2", target_bir_lowering=False, debug=False,
                       num_devices=N_CORES)
    finally:
        bass.BassGpSimd.memset = orig_memset
    d_main = nc.dram_tensor("main", [49, 288], F16, kind="ExternalInput")
    d_cv2 = nc.dram_tensor("cv2", [33, 1024], F16, kind="ExternalInput")
    d_hop = nc.dram_tensor("hop", [128, 1280], F16, kind="ExternalInput")
    d_wB = nc.dram_tensor("wB", [128, 1784], F16, kind="ExternalInput")
    d_wC = nc.dram_tensor("wC", [128, 1792], F16, kind="ExternalInput")
    d_smpl = nc.dram_tensor("smpl", [100, 192], F16, kind="ExternalInput")
    out_t = nc.dram_tensor("out", [64, 65], F16, kind="ExternalOutput")

    with tile.TileContext(nc) as tc:
        with tc.tile_pool(name="sb", bufs=1) as sb, \
             tc.tile_pool(name="ps", bufs=1, space="PSUM") as ps:
            # ---- PE warm-up ASAP: pe_busy_start anchors the p-state ramp;
            # full speed arrives 3us after the first PE instruction.  The
            # Bacc-preamble const tile is memset pre-barrier, so the warm
            # matmuls can fire the moment the start barrier clears.
            c1 = nc.const_aps.aps[(F32, 1.0)]
            for w_ in range(3):
                warm_ps = ps.tile([8, 8], F32, tag="q64", bufs=2,
                                  name=f"warm{w_}")
                nc.tensor.matmul(warm_ps[0:1, 0:1], c1, c1,
                                 start=True, stop=True)
            # Dummy activation with no data deps: LoadActFuncSet is inserted
            # before the FIRST activation and inherits its position's waits,
            # so give it one that can run immediately after the barrier.
            warm_a = sb.tile([128, 1], F16, tag="warm_a")
            nc.scalar.activation(out=warm_a[:], in_=c1, func=AF.Relu,
                                 bias=0.0, scale=1.0)

            # ---- input DMAs, ordered by first use (HWDGE serializes)
            # main/hop/smpl go via SP+HWDGE; cv2/wB/wC via Pool's SWDGE path.
            # scalar.dma_start would hold the ACT sequencer ~1.3us per DMA
            # (blocking relu/exp work), and splitting across the two DGE
            # paths also halves the descriptor-generation queue.
            main = sb.tile([49, 288], F16, tag="main")
            nc.sync.dma_start(out=main[:], in_=d_main[:])
            cv2 = sb.tile([33, 1024], F16, tag="cv2")
            nc.gpsimd.dma_start(out=cv2[:], in_=d_cv2[:])
            hop = sb.tile([128, 1280], F16, tag="hop")
            nc.sync.dma_start(out=hop[:], in_=d_hop[:])
            wB = sb.tile([128, 1784], F16, tag="wB")
            nc.gpsimd.dma_start(out=wB[:], in_=d_wB[:])
            smpl = sb.tile([100, 192], F16, tag="smpl")
            nc.sync.dma_start(out=smpl[:], in_=d_smpl[:])
            wC = sb.tile([128, 1792], F16, tag="wC")
            nc.gpsimd.dma_start(out=wC[:], in_=d_wC[:])
            # a1p zero/ones fills on DVE: Pool's engine is held ~1us per
            # SWDGE prep above, and conv1's relu needs a1p early.
            a1p = sb.tile([33, 18, 18], F16, tag="a1p")
            nc.vector.memset(a1p[0:32, :, :], 0.0)
            nc.vector.memset(a1p[32:33, :, :], 1.0)

            P1 = main[0:49, 0:256]
            w1f = main[0:49, 256:288]
            w2ta = cv2[:].rearrange("c (i o) -> c i o", i=16)
            w2fT = wC[:, 1536:1792].rearrange("k (t c) -> k t c", t=4)
            KT = hop[0:64, 0:512]
            w2b = hop[0:64, 768:1280]
            KV = hop[:, 512:768].rearrange("k (t c) -> k t c", t=4)
            Scomb = wB[:, 0:400].rearrange("k (t u) -> k t u", t=4)
            CandM = wB[0:100, 400:784].rearrange("u (c k) -> u c k", c=3)
            PermB = wB[0:16, 784:1684]
            NegI = wB[0:100, 1684:1784]
            W1big = wC[0:100, 0:1536].rearrange("u (h t k) -> u h t k",
                                                h=3, t=4)
            X = smpl[:].rearrange("u (h q) -> u h q", h=3)

            # ---- Pool: constants + zero-fills, all off the critical path
            # -1/64 column: the s-sum matmuls contract against it so the
            # PSUM row is -s/64 and a plain reciprocal yields -64/s.
            neg_col = sb.tile([128, 1], F16, tag="neg_col")
            nc.gpsimd.memset(neg_col[:], -1.0 / 64.0)
            cstk = sb.tile([16, 8, 8, 9], F16, tag="cstk")
            nc.gpsimd.memset(cstk[:], 0.0)
            eB = sb.tile([16, 12, 8], F16, tag="eB")
            nc.gpsimd.memset(eB[:], 0.0)

            # ---- conv1 (bias folded) + relu into padded a1p, split in two
            # p-row groups so conv2 can start before the relu finishes
            # Separate PSUM tiles per half: cross-engine readers of one PSUM
            # tile get serialized by the dep tracker, and the halves relu on
            # DVE (rows 0:10, feeds conv2 p 0:4) and ACT (rows 10:16) in
            # parallel.
            a1_ps = ps.tile([32, 160], F32, tag="a1", bufs=1)
            a1_psB = ps.tile([32, 96], F32, tag="gz", bufs=1)
            nc.tensor.matmul(a1_ps[:], w1f, P1[:, 0:160],
                             start=True, stop=True)
            nc.tensor.matmul(a1_psB[:], w1f, P1[:, 160:256],
                             start=True, stop=True)
            nc.vector.tensor_scalar(
                out=a1p[0:32, 1:11, 1:17],
                in0=a1_ps[:].rearrange("c (p q) -> c p q", p=10),
                scalar1=0.0, scalar2=None, op0=ALU.max)
            nc.scalar.activation(
                out=a1p[0:32, 11:17, 1:17],
                in_=a1_psB[:].rearrange("c (p q) -> c p q", p=6),
                func=AF.Relu, bias=0.0, scale=1.0)

            # ---- conv2 + relu directly from strided a1p windows:
            # rhs(t,a)[c1, p, q] = a1p[c1, 2p+t, 2q+a]; partition 32 of a1p
            # is all-ones so block (0,0)'s row 32 adds b2.
            # Split into p 0:4 (needs a1p rows 0:11) and p 4:8 (rows 8:17).
            a1p_ap = a1p[:]
            z2_ps = ps.tile([64, 64], F32, tag="q64", bufs=2)
            for half in range(2):
                po = half * 4
                i = 0
                for t in range(4):
                    for a in range(4):
                        rhs = bass.AP(
                            tensor=a1p_ap.tensor,
                            offset=a1p_ap.offset + (t + 2 * po) * 18 + a,
                            ap=[[324, 33], [36, 4], [2, 8]])
                        nc.tensor.matmul(
                            z2_ps[:, po * 8:(po + 4) * 8],
                            w2ta[:, t * 4 + a, :], rhs,
                            start=(i == 0), stop=(i == 15))
                        i += 1
            z2 = sb.tile([64, 64], F16, tag="z2")
            nc.scalar.activation(out=z2[:], in_=z2_ps[:], func=AF.Relu,
                                 bias=0.0, scale=1.0)

            # ---- relu-derivative masks, off the critical path:
            # M1W[a*32+c1, t, pq] = (a1p[c1, 2p+t, 2q+a] != 0)
            M1W = sb.tile([128, 4, 64], F16, tag="M1W")
            for a in range(4):
                src = bass.AP(
                    tensor=a1p_ap.tensor,
                    offset=a1p_ap.offset + a,
                    ap=[[324, 32], [18, 4], [36, 8], [2, 8]])
                dst = M1W[a * 32:(a + 1) * 32, :, :].rearrange(
                    "k t (p q) -> k t p q", p=8)
                nc.vector.tensor_scalar(out=dst, in0=src, scalar1=0.0,
                                        scalar2=None, op0=ALU.not_equal)
            m2 = sb.tile([64, 64], F16, tag="m2")
            nc.vector.tensor_scalar(out=m2[:], in0=z2[:], scalar1=0.0,
                                    scalar2=None, op0=ALU.not_equal)

            # ---- Hopfield #1 scores in S^T layout [m(4x128), pq]
            ST = ps.tile([128, 256], F32, tag="S", bufs=2, name="ST1")
            for t in range(4):
                nc.tensor.matmul(ST[:, t * 64:(t + 1) * 64],
                                 KT[:, t * 128:(t + 1) * 128], z2[:],
                                 start=True, stop=True)
            # Gz = w2b^T @ z2 runs in the softmax window (PE otherwise idle)
            gz_ps = ps.tile([128, 256], F32, tag="gz", bufs=1)
            for t in range(4):
                nc.tensor.matmul(gz_ps[:, t * 64:(t + 1) * 64],
                                 w2b[:, t * 128:(t + 1) * 128], z2[:],
                                 start=True, stop=True)
            att = sb.tile([128, 256], F16, tag="att1", name="att1")
            nc.scalar.activation(out=att[:], in_=ST[:], func=AF.Exp,
                                 bias=0.0, scale=BETA)
            # Gzm = Gz . M1W, then Cz = Scomb^T @ Gzm opens the C PSUM
            # accumulation group (Cq closes it below).
            Gzm = sb.tile([128, 4, 64], F16, tag="Gzm")
            nc.vector.tensor_tensor(
                out=Gzm[:].rearrange("k t u -> k (t u)"), in0=gz_ps[:],
                in1=M1W[:].rearrange("k t u -> k (t u)"), op=ALU.mult)

            # softmax denominators s then unnormalized q~ (KV/64 scale)
            s_ps = ps.tile([1, 64], F32, tag="srow", bufs=1)
            for t in range(4):
                nc.tensor.matmul(s_ps[:], neg_col[:],
                                 att[:, t * 64:(t + 1) * 64],
                                 start=(t == 0), stop=(t == 3))
            q_ps = ps.tile([64, 64], F32, tag="q64", bufs=2, name="q1")
            for t in range(4):
                nc.tensor.matmul(q_ps[:], KV[:, t, :],
                                 att[:, t * 64:(t + 1) * 64],
                                 start=(t == 0), stop=(t == 3))
            C_ps = ps.tile([100, 64], F32, tag="a1", bufs=1, name="Cps")
            for t in range(4):
                nc.tensor.matmul(C_ps[:], Scomb[:, t, :], Gzm[:, t, :],
                                 start=(t == 0), stop=False)

            # r = -64/s = 1/(-s/64), broadcast to 128 partitions on Pool,
            # folded into M1W -> M1WR (pbcast/M1WR are off the qm2/Gq spine)
            r1 = sb.tile([1, 64], F16, tag="r1")
            with nc.allow_low_precision(reason="softmax 1/sum in fp16"):
                nc.vector.reciprocal(r1[:], s_ps[:])
            rb128 = sb.tile([128, 64], F16, tag="rb128")
            nc.gpsimd.partition_broadcast(rb128[:], r1[:])
            qm2 = sb.tile([64, 64], F16, tag="qm2")
            nc.vector.tensor_tensor(out=qm2[:], in0=q_ps[:], in1=m2[:],
                                    op=ALU.mult)
            M1WR = sb.tile([128, 4, 64], F16, tag="M1WR")
            rb_b = bass.AP(tensor=rb128[:].tensor, offset=rb128[:].offset,
                           ap=[[rb128[:].ap[0][0], 128], [0, 4], [1, 64]])
            nc.vector.tensor_tensor(
                out=M1WR[:].rearrange("k t u -> k (t u)"),
                in0=M1W[:].rearrange("k t u -> k (t u)"), in1=rb_b,
                op=ALU.mult)

            # backward q-branch: Gq = w2b^T @ qm2, Gqm = Gq . M1WR (carries
            # -64/s), Cq accumulates into C_ps closing the group
            gq_ps = ps.tile([128, 256], F32, tag="S", bufs=2, name="gq")
            for t in range(4):
                nc.tensor.matmul(gq_ps[:, t * 64:(t + 1) * 64],
                                 w2b[:, t * 128:(t + 1) * 128], qm2[:],
                                 start=True, stop=True)
            Gqm = sb.tile([128, 4, 64], F16, tag="Gqm")
            nc.vector.tensor_tensor(
                out=Gqm[:].rearrange("k t u -> k (t u)"), in0=gq_ps[:],
                in1=M1WR[:].rearrange("k t u -> k (t u)"), op=ALU.mult)
            for t in range(4):
                nc.tensor.matmul(C_ps[:], Scomb[:, t, :], Gqm[:, t, :],
                                 start=False, stop=(t == 3))
            C16 = sb.tile([100, 64], F16, tag="C16")
            nc.vector.tensor_copy(out=C16[:], in_=C_ps[:])

            # ---- blocked e_min: 3 candidate matmuls into one PSUM tile,
            # one fp16 staging copy, 9 shifted stack copies, min-reduce
            # candPa/candPc are separate tiles because the tile framework
            # serializes cross-engine READERS of one PSUM tile: DVE stages
            # candPa; ACT's lone aligned copy reads candPc independently.
            candPa = ps.tile([128, 2, 64], F32, tag="g128", bufs=1)
            candPc = ps.tile([128, 64], F32, tag="gz", bufs=1)
            for cc in range(2):
                nc.tensor.matmul(candPa[:, cc, :], CandM[:, cc, :], C16[:],
                                 start=True, stop=True)
            nc.tensor.matmul(candPc[:], CandM[:, 2, :], C16[:],
                             start=True, stop=True)
            candS = sb.tile([128, 2, 8, 8], F16, tag="candS")
            nc.vector.tensor_copy(
                out=candS[:].rearrange("k c p q -> k (c p q)"),
                in_=candPa[:].rearrange("k c u -> k (c u)"))

            def stk_aps(k, j, fp16):
                cc, kk = divmod(k, 4)
                dp, dq = k // 3 - 1, k % 3 - 1
                i4lo, i4hi = max(0, dp), min(8, 8 + dp)
                j4lo, j4hi = max(0, dq), min(8, 8 + dq)
                if fp16:
                    src = candS[kk * 32:kk * 32 + 16, cc,
                                i4lo - dp:i4hi - dp, j4lo - dq:j4hi - dq,
                                None]
                else:
                    src = candPc[kk * 32:kk * 32 + 16, :].rearrange(
                        "m (p q) -> m p q", p=8)[:, i4lo - dp:i4hi - dp,
                                                 j4lo - dq:j4hi - dq, None]
                return src, cstk[:, i4lo:i4hi, j4lo:j4hi, j:j + 1]
            # ACT: the cc2 candidate straight from its own PSUM tile, plus
            # one from the fp16 stage (SBUF readers do not serialize)
            src, dst = stk_aps(8, 7, False)
            nc.scalar.copy(out=dst, in_=src)
            src, dst = stk_aps(1, 1, True)
            nc.scalar.copy(out=dst, in_=src)
            # DVE: 5 aligned fp16 copies from the staged tile
            for j, k in [(0, 0), (2, 2), (3, 3), (4, 5), (8, 4)]:
                src, dst = stk_aps(k, j, True)
                nc.vector.tensor_copy(out=dst, in_=src)
            # Pool: 2 aligned fp16 copies
            for j, k in [(5, 6), (6, 7)]:
                src, dst = stk_aps(k, j, True)
                nc.gpsimd.tensor_copy(out=dst, in_=src)
            # Reduce in two row groups: the pq-half-A eW windows only read
            # eB rows 2:8, so half A's gather starts after the first part.
            nc.vector.tensor_reduce(out=eB[:, 2:8, :], in_=cstk[:, 0:6, :, :],
                                    axis=mybir.AxisListType.X, op=ALU.min)
            nc.vector.tensor_reduce(out=eB[:, 8:10, :], in_=cstk[:, 6:8, :, :],
                                    axis=mybir.AxisListType.X, op=ALU.min)

            # ---- eW gather (one-hot PermB), then subtract C16 in PSUM via
            # the -I matmul: D = eW - C16 >= 0 iff mask=1 (exact: both are
            # fp16 values differenced in fp32), so the masked input is one
            # fused op Xm = (D >= 0) * X.
            #
            # Everything from here to Hopfield #2 is software-pipelined in
            # two pq-column halves: half B's PE stages run under half A's
            # DVE/ACT stages.  Ops are emitted A/B-interleaved per engine so
            # the in-order sequencers never block a half on the other's
            # later stages.
            eBf = eB[:].rearrange("a b c -> a (b c)")
            D_half = [ps.tile([100, 32], F32, tag="a1", bufs=1, name="eWa"),
                      ps.tile([100, 32], F32, tag="gz", bufs=1, name="eWb")]
            for h2 in range(2):
                co = 32 * h2
                # -C16 first: only needs C16, runs during the min-reduce
                nc.tensor.matmul(D_half[h2][:], NegI, C16[:, co:co + 32],
                                 start=True, stop=False)
            for h2 in range(2):
                co = 32 * h2
                for k in range(9):
                    dp, dq = k // 3 - 1, k % 3 - 1
                    off = 16 + 8 * dp + dq + co
                    nc.tensor.matmul(D_half[h2][:],
                                     PermB[:, k * 100:(k + 1) * 100],
                                     eBf[:, off:off + 32],
                                     start=False, stop=(k == 8))
            Xm = sb.tile([100, 3, 64], F16, tag="Xm")
            for h2 in range(2):
                co = 32 * h2
                dap = D_half[h2][:]
                D_b = bass.AP(tensor=dap.tensor, offset=dap.offset,
                              ap=[[dap.ap[0][0], 100], [0, 3], [1, 32]])
                nc.vector.scalar_tensor_tensor(
                    out=Xm[:, :, co:co + 32], in0=D_b, scalar=0.0,
                    in1=X[:, :, co:co + 32], op0=ALU.is_ge, op1=ALU.mult)
            u1_ps = [ps.tile([128, 4, 32], F32, tag="S", bufs=2, name="u1a"),
                     ps.tile([128, 4, 32], F32, tag="S", bufs=2, name="u1b")]
            for h2 in range(2):
                co = 32 * h2
                for t in range(4):
                    for h in range(3):
                        nc.tensor.matmul(u1_ps[h2][:, t, :],
                                         W1big[:, h, t, :],
                                         Xm[:, h, co:co + 32],
                                         start=(h == 0), stop=(h == 2))
            u1m = sb.tile([128, 4, 64], F16, tag="u1m")
            for h2 in range(2):
                co = 32 * h2
                nc.vector.tensor_tensor(
                    out=u1m[:, :, co:co + 32], in0=u1_ps[h2][:],
                    in1=M1W[:].rearrange("k t (x u) -> k t x u",
                                         x=2)[:, :, h2, :], op=ALU.mult)
            zm_ps = [ps.tile([64, 32], F32, tag="q64", bufs=2, name="zma"),
                     ps.tile([64, 32], F32, tag="q64", bufs=2, name="zmb")]
            for h2 in range(2):
                co = 32 * h2
                for t in range(4):
                    nc.tensor.matmul(zm_ps[h2][:], w2fT[:, t, :],
                                     u1m[:, t, co:co + 32],
                                     start=(t == 0), stop=(t == 3))
            z2m = sb.tile([64, 64], F16, tag="z2m")
            for h2 in range(2):
                co = 32 * h2
                nc.vector.tensor_tensor(out=z2m[:, co:co + 32],
                                        in0=zm_ps[h2][:],
                                        in1=m2[:, co:co + 32], op=ALU.mult)

            # ---- Hopfield #2 -> ship q2/64 (cols 0:64) and s2 (col 64);
            # the host computes out = -q~2/s2' with s2' = -s2/64.
            ST2 = [ps.tile([128, 4, 32], F32, tag="S", bufs=2, name="ST2a"),
                   ps.tile([128, 4, 32], F32, tag="S", bufs=2, name="ST2b")]
            for h2 in range(2):
                co = 32 * h2
                for t in range(4):
                    nc.tensor.matmul(ST2[h2][:, t, :],
                                     KT[:, t * 128:(t + 1) * 128],
                                     z2m[:, co:co + 32],
                                     start=True, stop=True)
            att2 = sb.tile([128, 4, 64], F16, tag="att2", name="att2")
            for h2 in range(2):
                co = 32 * h2
                nc.scalar.activation(out=att2[:, :, co:co + 32],
                                     in_=ST2[h2][:], func=AF.Exp,
                                     bias=0.0, scale=BETA)
            qs2 = ps.tile([64, 65], F32, tag="q64", bufs=2, name="qs2")
            for h2 in range(2):
                co = 32 * h2
                for t in range(4):
                    nc.tensor.matmul(qs2[:, co:co + 32], KV[:, t, :],
                                     att2[:, t, co:co + 32],
                                     start=(t == 0), stop=(t == 3))
            for t in range(4):
                nc.tensor.matmul(qs2[:, 64:65], att2[:, t, :],
                                 neg_col[:], start=(t == 0), stop=(t == 3))
            out_sb = sb.tile([64, 65], F16, tag="out_sb")
            nc.vector.tensor_copy(out=out_sb[:, 0:32], in_=qs2[:, 0:32])
            nc.vector.tensor_copy(out=out_sb[:, 32:65], in_=qs2[:, 32:65])
            nc.sync.dma_start(out=out_t[:], in_=out_sb[:])
    nc.compile()
    return nc


def _get_nc(debug=False):
    key = ("nc", debug)
    if key not in _CACHE:
        _CACHE[key] = _build_nc(debug)
    return _CACHE[key]


# ---------------------------------------------------------------- entry point
def kernel(x, w1, b1, w2, b2, K, Vw, _debug=False):
    x = np.asarray(x, np.float32)
    shared = _host_prep(np.asarray(w1, np.float32), np.asarray(b1, np.float32),
                        np.asarray(w2, np.float32), np.asarray(b2, np.float32),
                        np.asarray(K, np.float32), np.asarray(Vw, np.float32))
    bsz = x.shape[0]
    nc = _get_nc(False)
    smpls = [_sample_prep(x[b]) for b in range(bsz)]
    in_maps = []
    for core in range(N_CORES):
        P1b, Xb = smpls[core] if core < bsz else smpls[0]
        mainb = shared["main"].copy()
        mainb[0:49, 0:256] = P1b
        m = {"main": mainb, "cv2": shared["cv2"], "hop": shared["hop"],
             "wB": shared["wB"], "wC": shared["wC"], "smpl": Xb}
        in_maps.append(m)
    res = run_bass_kernel_spmd(nc, in_maps, core_ids=list(range(N_CORES)))
    outs = []
    for b in range(bsz):
        r = np.asarray(res.results[b]["out"], np.float32)
        # col 64 holds -s2/64, so q2_true = -q~2 / (-s2/64)
        outs.append((-r[:, 0:64] / r[:, 64:65].T).reshape(64, 8, 8))
    out = np.stack(outs).astype(np.float32)
    if _debug:
        return out, res
    return out


# revision 61
# speedup vs baseline: 1.0172x; 1.0172x over previous
"""TRN2 Bass kernel for nn_Block1_43542378447225 (v2, latency-tuned).

Per sample on one NeuronCore (batch=2 -> cores 0/1 do real work):
  conv1 (bias folded into matmul via ones row) -> relu into padded a1p
  -> conv2 (split p-halves; b2 folded via all-ones a1p partition 32)
  -> z2 -> Hopfield #1 in S^T layout -> backward split C = Cz - Cq:
       Cz = Scomb^T((w2b^T z2) . M1W)  [runs during hop1 softmax window]
       Cq = Scomb^T((w2b^T (q~ . m2)) . M1WR)
     where M1WR = M1W * (-64/s) folds the softmax normalization (a per-
     column scale commutes through both partition-contracting matmuls)
     and Cq accumulates into Cz's PSUM tile, so C needs one PSUM->SBUF
     convert -> blocked e_min (single staged cand tile + shifted stack +
     min-reduce) -> eW gather -> mask -> masked patch forward (W1big)
  -> z2_masked -> Hopfield #2 -> out (host divides q2*64/s2).

All SBUF data fp16 (PE 4x faster than fp32; DVE 2x/4x modes need all-
fp16-SBUF operands); PSUM fp32.  The C -> e_min -> mask comparison path
stays bit-exact in fp16: cand/eW matmuls are one-hot gathers, so every
candidate equals an fp16-rounded C entry exactly and the argmin survives
`C16 <= eW`.

Layout: pq = p*8+q (64 conv2 output positions), uv = u*10+v (100
composite window offsets), kc = a*32 + c1 (hidden index; chunk t = conv2
kernel row, a = conv2 kernel col).
"""
import numpy as np

import concourse.bass as bass
import concourse.bacc as bacc
import concourse.mybir as mybir
import concourse.tile as tile
from concourse.tile import add_dep_helper
from concourse.bass_utils import run_bass_kernel_spmd

F32 = mybir.dt.float32
F16 = mybir.dt.float16
AF = mybir.ActivationFunctionType
ALU = mybir.AluOpType

N_CORES = 8
BETA = 0.125  # 1/sqrt(64)

_CACHE = {}


# ---------------------------------------------------------------- host prep
def _build_scomb_w1big(w1):
    w1s = w1.sum(axis=1)
    Scomb = np.zeros((4, 32, 4, 100), np.float32)  # [a, c1, t, uv]
    W1big = np.zeros((100, 3, 4, 4, 32), np.float32)  # [uv, h, t, a, c1]
    for t in range(4):
        for a in range(4):
            for u in range(10):
                ki = u - 2 * t
                if not (0 <= ki < 4):
                    continue
                for v in range(10):
                    kj = v - 2 * a
                    if not (0 <= kj < 4):
                        continue
                    Scomb[a, :, t, u * 10 + v] = w1s[:, ki, kj]
                    W1big[u * 10 + v, :, t, a, :] = w1[:, :, ki, kj].T
    # partition index = a*32+c1 -> merge (a, c1); free = t*100+uv
    return Scomb.reshape(128, 400), W1big.reshape(100, 1536)


def _host_prep(w1, b1, w2, b2, K, Vw):
    # cv1 template [49, 288]: per-sample P1 (cols 0:256, row 48 = ones)
    # filled later; w1f cols 256:288 with b1 folded into row 48.
    main = np.zeros((49, 288), np.float16)
    main[0:48, 256:288] = np.transpose(w1, (2, 3, 1, 0)).reshape(48, 32)
    main[48, 256:288] = b1

    # conv2 weights, every (t, a) block based at partition 0:
    # cv2[c1, (t*4+a)*64 + o] = w2[o, c1, t, a]; b2 folded into partition
    # 32 of block (0,0) -- a1p partition 32 is all-ones on device.
    cv2 = np.zeros((33, 1024), np.float16)
    cv2[0:32, :] = np.transpose(w2, (1, 2, 3, 0)).reshape(32, 1024)
    cv2[32, 0:64] = b2

    # hop [128, 1280]: KT | KV chunks | w2b (2.0 folded, rows 0:64 -- matmul
    # lhsT must share the rhs base partition).  Shipping w2b here (instead
    # of wB) lets the Gz matmuls run inside the hop1 softmax window.
    hop = np.zeros((128, 1280), np.float16)
    hop[0:64, 0:512] = K.T
    # KV chunks [128, 4, 64]: KV[m, e] = (K @ Vw)[m, e] / 64 (the 1/64 keeps
    # the unnormalized q inside fp16 range; -64/s is folded back via M1WR)
    KVh = (K @ Vw).astype(np.float32).reshape(4, 128, 64) / 64.0
    hop[:, 512:768] = np.transpose(KVh, (1, 0, 2)).reshape(128, 256)
    hop[0:64, 768:1280] = 2.0 * np.transpose(w2, (0, 2, 3, 1)).reshape(64, 512)

    Scomb, W1big = _build_scomb_w1big(w1)
    PermF = np.zeros((100, 9, 16), np.float32)
    for k in range(9):
        dp, dq = k // 3 - 1, k % 3 - 1
        for im in range(4):
            u = 4 * dp + im + 3
            if not (0 <= u < 10):
                continue
            for jm in range(4):
                v = 4 * dq + jm + 3
                if not (0 <= v < 10):
                    continue
                PermF[u * 10 + v, k, im * 4 + jm] = 1.0
    CandM = np.zeros((100, 3, 128), np.float32)
    for k in range(9):
        cc, kk = divmod(k, 4)
        CandM[:, cc, kk * 32:kk * 32 + 16] = PermF[:, k, :]
    PermB = np.transpose(PermF, (2, 1, 0)).reshape(16, 900)

    # wB [128, 1784]: Scomb | CandM | PermB | -I (the eW accumulation
    # subtracts C16 so the mask compare fuses into one scalar_tensor_tensor)
    wB = np.zeros((128, 1784), np.float16)
    wB[:, 0:400] = Scomb
    wB[0:100, 400:784] = CandM.reshape(100, 384)
    wB[0:16, 784:1684] = PermB
    wB[0:100, 1684:1784] = -np.eye(100, dtype=np.float16)

    wC = np.zeros((128, 1792), np.float16)
    wC[0:100, 0:1536] = W1big
    wC[:, 1536:1792] = np.transpose(w2, (3, 1, 2, 0)).reshape(128, 256)
    return {"main": main, "cv2": cv2, "hop": hop, "wB": wB, "wC": wC}


def _sample_prep(x_s):
    xp1 = np.pad(x_s, ((0, 0), (1, 1), (1, 1)))
    xp3 = np.pad(x_s, ((0, 0), (3, 3), (3, 3)))
    P1 = np.zeros((4, 4, 3, 16, 16), np.float32)
    for kr in range(4):
        for ks in range(4):
            P1[kr, ks] = xp1[:, kr:kr + 32:2, ks:ks + 32:2][:, :16, :16]
    X = np.zeros((10, 10, 3, 8, 8), np.float32)
    for u in range(10):
        for v in range(10):
            X[u, v] = xp3[:, u:u + 32:4, v:v + 32:4][:, :8, :8]
    P1f = np.ones((49, 256), np.float32)
    P1f[0:48] = P1.reshape(48, 256)
    return (P1f.astype(np.float16), X.reshape(100, 192).astype(np.float16))


# ---------------------------------------------------------------- device build
def _build_nc(debug=False):
    # Route two of the four Bacc-preamble const memsets (all Pool) to DVE
    # so they parallelize and the start barrier clears ~200ns earlier.
    orig_memset = bass.BassGpSimd.memset
    state = {"n": 0}

    def routed_memset(self, ap, constant):
        state["n"] += 1
        if state["n"] in (2, 3, 4):
            return bass.BassVectorEngine.memset(self.bass.vector, ap, constant)
        return orig_memset(self, ap, constant)

    bass.BassGpSimd.memset = routed_memset
    try:
        nc = bacc.Bacc("TRN2", target_bir_lowering=False, debug=False,
                       num_devices=N_CORES)
    finally:
        bass.BassGpSimd.memset = orig_memset
    d_main = nc.dram_tensor("main", [49, 288], F16, kind="ExternalInput")
    d_cv2 = nc.dram_tensor("cv2", [33, 1024], F16, kind="ExternalInput")
    d_hop = nc.dram_tensor("hop", [128, 1280], F16, kind="ExternalInput")
    d_wB = nc.dram_tensor("wB", [128, 1784], F16, kind="ExternalInput")
    d_wC = nc.dram_tensor("wC", [128, 1792], F16, kind="ExternalInput")
    d_smpl = nc.dram_tensor("smpl", [100, 192], F16, kind="ExternalInput")
    out_t = nc.dram_tensor("out", [64, 65], F16, kind="ExternalOutput")

    with tile.TileContext(nc) as tc:
        with tc.tile_pool(name="sb", bufs=1) as sb, \
             tc.tile_pool(name="ps", bufs=1, space="PSUM") as ps:
            # ---- PE warm-up ASAP: pe_busy_start anchors the p-state ramp;
            # full speed arrives 3us after the first PE instruction.  The
            # Bacc-preamble const tile is memset pre-barrier, so the warm
            # matmuls can fire the moment the start barrier clears.
            c1 = nc.const_aps.aps[(F32, 1.0)]
            for w_ in range(3):
                warm_ps = ps.tile([8, 8], F32, tag="q64", bufs=2,
                                  name=f"warm{w_}")
                nc.tensor.matmul(warm_ps[0:1, 0:1], c1, c1,
                                 start=True, stop=True)
            # Dummy activation with no data deps: LoadActFuncSet is inserted
            # before the FIRST activation and inherits its position's waits,
            # so give it one that can run immediately after the barrier.
            warm_a = sb.tile([128, 1], F16, tag="warm_a")
            nc.scalar.activation(out=warm_a[:], in_=c1, func=AF.Relu,
                                 bias=0.0, scale=1.0)

            # ---- input DMAs, ordered by first use (HWDGE serializes)
            # main/hop/smpl go via SP+HWDGE; cv2/wB/wC via Pool's SWDGE path.
            # scalar.dma_start would hold the ACT sequencer ~1.3us per DMA
            # (blocking relu/exp work), and splitting across the two DGE
            # paths also halves the descriptor-generation queue.
            main = sb.tile([49, 288], F16, tag="main")
            nc.sync.dma_start(out=main[:], in_=d_main[:])
            cv2 = sb.tile([33, 1024], F16, tag="cv2")
            nc.gpsimd.dma_start(out=cv2[:], in_=d_cv2[:])
            hop = sb.tile([128, 1280], F16, tag="hop")
            nc.sync.dma_start(out=hop[:], in_=d_hop[:])
            wB = sb.tile([128, 1784], F16, tag="wB")
            nc.gpsimd.dma_start(out=wB[:], in_=d_wB[:])
            smpl = sb.tile([100, 192], F16, tag="smpl")
            nc.sync.dma_start(out=smpl[:], in_=d_smpl[:])
            wC = sb.tile([128, 1792], F16, tag="wC")
            nc.gpsimd.dma_start(out=wC[:], in_=d_wC[:])
            # a1p zero/ones fills on DVE: Pool's engine is held ~1us per
            # SWDGE prep above, and conv1's relu needs a1p early.
            a1p = sb.tile([33, 18, 18], F16, tag="a1p")
            nc.vector.memset(a1p[0:32, :, :], 0.0)
            nc.vector.memset(a1p[32:33, :, :], 1.0)

            P1 = main[0:49, 0:256]
            w1f = main[0:49, 256:288]
            w2ta = cv2[:].rearrange("c (i o) -> c i o", i=16)
            w2fT = wC[:, 1536:1792].rearrange("k (t c) -> k t c", t=4)
            KT = hop[0:64, 0:512]
            w2b = hop[0:64, 768:1280]
            KV = hop[:, 512:768].rearrange("k (t c) -> k t c", t=4)
            Scomb = wB[:, 0:400].rearrange("k (t u) -> k t u", t=4)
            CandM = wB[0:100, 400:784].rearrange("u (c k) -> u c k", c=3)
            PermB = wB[0:16, 784:1684]
            NegI = wB[0:100, 1684:1784]
            W1big = wC[0:100, 0:1536].rearrange("u (h t k) -> u h t k",
                                                h=3, t=4)
            X = smpl[:].rearrange("u (h q) -> u h q", h=3)

            # ---- Pool: constants + zero-fills, all off the critical path
            # -1/64 column: the s-sum matmuls contract against it so the
            # PSUM row is -s/64 and a plain reciprocal yields -64/s.
            neg_col = sb.tile([128, 1], F16, tag="neg_col")
            nc.gpsimd.memset(neg_col[:], -1.0 / 64.0)
            cstk = sb.tile([16, 8, 8, 9], F16, tag="cstk")
            nc.gpsimd.memset(cstk[:], 0.0)
            eB = sb.tile([16, 12, 8], F16, tag="eB")
            nc.gpsimd.memset(eB[:], 0.0)

            # ---- conv1 (bias folded) + relu into padded a1p, split in two
            # p-row groups so conv2 can start before the relu finishes
            # Separate PSUM tiles per half: cross-engine readers of one PSUM
            # tile get serialized by the dep tracker, and the halves relu on
            # DVE (rows 0:10, feeds conv2 p 0:4) and ACT (rows 10:16) in
            # parallel.
            a1_ps = ps.tile([32, 160], F32, tag="a1", bufs=1)
            a1_psB = ps.tile([32, 96], F32, tag="gz", bufs=1)
            nc.tensor.matmul(a1_ps[:], w1f, P1[:, 0:160],
                             start=True, stop=True)
            nc.tensor.matmul(a1_psB[:], w1f, P1[:, 160:256],
                             start=True, stop=True)
            nc.vector.tensor_scalar(
                out=a1p[0:32, 1:11, 1:17],
                in0=a1_ps[:].rearrange("c (p q) -> c p q", p=10),
                scalar1=0.0, scalar2=None, op0=ALU.max)
            nc.scalar.activation(
                out=a1p[0:32, 11:17, 1:17],
                in_=a1_psB[:].rearrange("c (p q) -> c p q", p=6),
                func=AF.Relu, bias=0.0, scale=1.0)

            # ---- conv2 + relu directly from strided a1p windows:
            # rhs(t,a)[c1, p, q] = a1p[c1, 2p+t, 2q+a]; partition 32 of a1p
            # is all-ones so block (0,0)'s row 32 adds b2.
            # Split into p 0:4 (needs a1p rows 0:11) and p 4:8 (rows 8:17).
            a1p_ap = a1p[:]
            z2_ps = ps.tile([64, 64], F32, tag="q64", bufs=2)
            for half in range(2):
                po = half * 4
                i = 0
                for t in range(4):
                    for a in range(4):
                        rhs = bass.AP(
                            tensor=a1p_ap.tensor,
                            offset=a1p_ap.offset + (t + 2 * po) * 18 + a,
                            ap=[[324, 33], [36, 4], [2, 8]])
                        nc.tensor.matmul(
                            z2_ps[:, po * 8:(po + 4) * 8],
                            w2ta[:, t * 4 + a, :], rhs,
                            start=(i == 0), stop=(i == 15))
                        i += 1
            z2 = sb.tile([64, 64], F16, tag="z2")
            nc.scalar.activation(out=z2[:], in_=z2_ps[:], func=AF.Relu,
                                 bias=0.0, scale=1.0)

            # ---- relu-derivative masks, off the critical path:
            # M1W[a*32+c1, t, pq] = (a1p[c1, 2p+t, 2q+a] != 0)
            M1W = sb.tile([128, 4, 64], F16, tag="M1W")
            for a in range(4):
                src = bass.AP(
                    tensor=a1p_ap.tensor,
                    offset=a1p_ap.offset + a,
                    ap=[[324, 32], [18, 4], [36, 8], [2, 8]])
                dst = M1W[a * 32:(a + 1) * 32, :, :].rearrange(
                    "k t (p q) -> k t p q", p=8)
                nc.vector.tensor_scalar(out=dst, in0=src, scalar1=0.0,
                                        scalar2=None, op0=ALU.not_equal)
            m2 = sb.tile([64, 64], F16, tag="m2")
            nc.vector.tensor_scalar(out=m2[:], in0=z2[:], scalar1=0.0,
                                    scalar2=None, op0=ALU.not_equal)

            # ---- Hopfield #1 scores in S^T layout [m(4x128), pq]
            ST = ps.tile([128, 256], F32, tag="S", bufs=2, name="ST1")
            for t in range(4):
                nc.tensor.matmul(ST[:, t * 64:(t + 1) * 64],
                                 KT[:, t * 128:(t + 1) * 128], z2[:],
                                 start=True, stop=True)
            # Gz = w2b^T @ z2 runs in the softmax window (PE otherwise idle)
            gz_ps = ps.tile([128, 256], F32, tag="gz", bufs=1)
            for t in range(4):
                nc.tensor.matmul(gz_ps[:, t * 64:(t + 1) * 64],
                                 w2b[:, t * 128:(t + 1) * 128], z2[:],
                                 start=True, stop=True)
            att = sb.tile([128, 256], F16, tag="att1", name="att1")
            nc.scalar.activation(out=att[:], in_=ST[:], func=AF.Exp,
                                 bias=0.0, scale=BETA)
            # Gzm = Gz . M1W, then Cz = Scomb^T @ Gzm opens the C PSUM
            # accumulation group (Cq closes it below).
            Gzm = sb.tile([128, 4, 64], F16, tag="Gzm")
            nc.vector.tensor_tensor(
                out=Gzm[:].rearrange("k t u -> k (t u)"), in0=gz_ps[:],
                in1=M1W[:].rearrange("k t u -> k (t u)"), op=ALU.mult)

            # softmax denominators s then unnormalized q~ (KV/64 scale)
            s_ps = ps.tile([1, 64], F32, tag="srow", bufs=1)
            for t in range(4):
                nc.tensor.matmul(s_ps[:], neg_col[:],
                                 att[:, t * 64:(t + 1) * 64],
                                 start=(t == 0), stop=(t == 3))
            q_ps = ps.tile([64, 64], F32, tag="q64", bufs=2, name="q1")
            for t in range(4):
                nc.tensor.matmul(q_ps[:], KV[:, t, :],
                                 att[:, t * 64:(t + 1) * 64],
                                 start=(t == 0), stop=(t == 3))
            C_ps = ps.tile([100, 64], F32, tag="a1", bufs=1, name="Cps")
            for t in range(4):
                nc.tensor.matmul(C_ps[:], Scomb[:, t, :], Gzm[:, t, :],
                                 start=(t == 0), stop=False)

            # r = -64/s = 1/(-s/64), broadcast to 128 partitions on Pool,
            # folded into M1W -> M1WR (pbcast/M1WR are off the qm2/Gq spine)
            r1 = sb.tile([1, 64], F16, tag="r1")
            with nc.allow_low_precision(reason="softmax 1/sum in fp16"):
                nc.vector.reciprocal(r1[:], s_ps[:])
            rb128 = sb.tile([128, 64], F16, tag="rb128")
            nc.gpsimd.partition_broadcast(rb128[:], r1[:])
            qm2 = sb.tile([64, 64], F16, tag="qm2")
            nc.vector.tensor_tensor(out=qm2[:], in0=q_ps[:], in1=m2[:],
                                    op=ALU.mult)
            M1WR = sb.tile([128, 4, 64], F16, tag="M1WR")
            rb_b = bass.AP(tensor=rb128[:].tensor, offset=rb128[:].offset,
                           ap=[[rb128[:].ap[0][0], 128], [0, 4], [1, 64]])
            nc.vector.tensor_tensor(
                out=M1WR[:].rearrange("k t u -> k (t u)"),
                in0=M1W[:].rearrange("k t u -> k (t u)"), in1=rb_b,
                op=ALU.mult)

            # backward q-branch: Gq = w2b^T @ qm2, Gqm = Gq . M1WR (carries
            # -64/s), Cq accumulates into C_ps closing the group
            gq_ps = ps.tile([128, 256], F32, tag="S", bufs=2, name="gq")
            for t in range(4):
                nc.tensor.matmul(gq_ps[:, t * 64:(t + 1) * 64],
                                 w2b[:, t * 128:(t + 1) * 128], qm2[:],
                                 start=True, stop=True)
            Gqm = sb.tile([128, 4, 64], F16, tag="Gqm")
            nc.vector.tensor_tensor(
                out=Gqm[:].rearrange("k t u -> k (t u)"), in0=gq_ps[:],
                in1=M1WR[:].rearrange("k t u -> k (t u)"), op=ALU.mult)
            for t in range(4):
                nc.tensor.matmul(C_ps[:], Scomb[:, t, :], Gqm[:, t, :],
                                 start=False, stop=(t == 3))
            C16 = sb.tile([100, 64], F16, tag="C16")
            nc.vector.tensor_copy(out=C16[:], in_=C_ps[:])

            # ---- blocked e_min: 3 candidate matmuls into one PSUM tile,
            # one fp16 staging copy, 9 shifted stack copies, min-reduce
            # candPa/candPc are separate tiles because the tile framework
            # serializes cross-engine READERS of one PSUM tile: DVE stages
            # candPa; ACT's lone aligned copy reads candPc independently.
            candPa = ps.tile([128, 2, 64], F32, tag="g128", bufs=1)
            candPc = ps.tile([128, 64], F32, tag="gz", bufs=1)
            for cc in range(2):
                nc.tensor.matmul(candPa[:, cc, :], CandM[:, cc, :], C16[:],
                                 start=True, stop=True)
            nc.tensor.matmul(candPc[:], CandM[:, 2, :], C16[:],
                             start=True, stop=True)
            candS = sb.tile([128, 2, 8, 8], F16, tag="candS")
            nc.vector.tensor_copy(
                out=candS[:].rearrange("k c p q -> k (c p q)"),
                in_=candPa[:].rearrange("k c u -> k (c u)"))

            def stk_aps(k, j, fp16):
                cc, kk = divmod(k, 4)
                dp, dq = k // 3 - 1, k % 3 - 1
                i4lo, i4hi = max(0, dp), min(8, 8 + dp)
                j4lo, j4hi = max(0, dq), min(8, 8 + dq)
                if fp16:
                    src = candS[kk * 32:kk * 32 + 16, cc,
                                i4lo - dp:i4hi - dp, j4lo - dq:j4hi - dq,
                                None]
                else:
                    src = candPc[kk * 32:kk * 32 + 16, :].rearrange(
                        "m (p q) -> m p q", p=8)[:, i4lo - dp:i4hi - dp,
                                                 j4lo - dq:j4hi - dq, None]
                return src, cstk[:, i4lo:i4hi, j4lo:j4hi, j:j + 1]
            # ACT: the cc2 candidate straight from its own PSUM tile, plus
            # one from the fp16 stage (SBUF readers do not serialize)
            src, dst = stk_aps(8, 7, False)
            nc.scalar.copy(out=dst, in_=src)
            src, dst = stk_aps(1, 1, True)
            nc.scalar.copy(out=dst, in_=src)
            # DVE: 5 aligned fp16 copies from the staged tile
            for j, k in [(0, 0), (2, 2), (3, 3), (4, 5), (8, 4)]:
                src, dst = stk_aps(k, j, True)
                nc.vector.tensor_copy(out=dst, in_=src)
            # Pool: 2 aligned fp16 copies
            for j, k in [(5, 6), (6, 7)]:
                src, dst = stk_aps(k, j, True)
                nc.gpsimd.tensor_copy(out=dst, in_=src)
            # Reduce in two row groups: the pq-half-A eW windows only read
            # eB rows 2:8, so half A's gather starts after the first part.
            nc.vector.tensor_reduce(out=eB[:, 2:8, :], in_=cstk[:, 0:6, :, :],
                                    axis=mybir.AxisListType.X, op=ALU.min)
            nc.vector.tensor_reduce(out=eB[:, 8:10, :], in_=cstk[:, 6:8, :, :],
                                    axis=mybir.AxisListType.X, op=ALU.min)

            # ---- eW gather (one-hot PermB), then subtract C16 in PSUM via
            # the -I matmul: D = eW - C16 >= 0 iff mask=1 (exact: both are
            # fp16 values differenced in fp32), so the masked input is one
            # fused op Xm = (D >= 0) * X.
            #
            # Everything from here to Hopfield #2 is software-pipelined in
            # two pq-column halves: half B's PE stages run under half A's
            # DVE/ACT stages.  Ops are emitted A/B-interleaved per engine so
            # the in-order sequencers never block a half on the other's
            # later stages.
            eBf = eB[:].rearrange("a b c -> a (b c)")
            D_half = [ps.tile([100, 32], F32, tag="a1", bufs=1, name="eWa"),
                      ps.tile([100, 32], F32, tag="gz", bufs=1, name="eWb")]
            for h2 in range(2):
                co = 32 * h2
                # -C16 first: only needs C16, runs during the min-reduce
                nc.tensor.matmul(D_half[h2][:], NegI, C16[:, co:co + 32],
                                 start=True, stop=False)
            for h2 in range(2):
                co = 32 * h2
                for k in range(9):
                    dp, dq = k // 3 - 1, k % 3 - 1
                    off = 16 + 8 * dp + dq + co
                    nc.tensor.matmul(D_half[h2][:],
                                     PermB[:, k * 100:(k + 1) * 100],
                                     eBf[:, off:off + 32],
                                     start=False, stop=(k == 8))
            Xm = sb.tile([100, 3, 64], F16, tag="Xm")
            for h2 in range(2):
                co = 32 * h2
                dap = D_half[h2][:]
                D_b = bass.AP(tensor=dap.tensor, offset=dap.offset,
                              ap=[[dap.ap[0][0], 100], [0, 3], [1, 32]])
                nc.vector.scalar_tensor_tensor(
                    out=Xm[:, :, co:co + 32], in0=D_b, scalar=0.0,
                    in1=X[:, :, co:co + 32], op0=ALU.is_ge, op1=ALU.mult)
            u1_ps = [ps.tile([128, 4, 32], F32, tag="S", bufs=2, name="u1a"),
                     ps.tile([128, 4, 32], F32, tag="S", bufs=2, name="u1b")]
            for h2 in range(2):
                co = 32 * h2
                for t in range(4):
                    for h in range(3):
                        nc.tensor.matmul(u1_ps[h2][:, t, :],
                                         W1big[:, h, t, :],
                                         Xm[:, h, co:co + 32],
                                         start=(h == 0), stop=(h == 2))
            u1m = sb.tile([128, 4, 64], F16, tag="u1m")
            for h2 in range(2):
                co = 32 * h2
                nc.vector.tensor_tensor(
                    out=u1m[:, :, co:co + 32], in0=u1_ps[h2][:],
                    in1=M1W[:].rearrange("k t (x u) -> k t x u",
                                         x=2)[:, :, h2, :], op=ALU.mult)
            zm_ps = [ps.tile([64, 32], F32, tag="q64", bufs=2, name="zma"),
                     ps.tile([64, 32], F32, tag="q64", bufs=2, name="zmb")]
            for h2 in range(2):
                co = 32 * h2
                for t in range(4):
                    nc.tensor.matmul(zm_ps[h2][:], w2fT[:, t, :],
                                     u1m[:, t, co:co + 32],
                                     start=(t == 0), stop=(t == 3))
            z2m = sb.tile([64, 64], F16, tag="z2m")
            for h2 in range(2):
                co = 32 * h2
                nc.vector.tensor_tensor(out=z2m[:, co:co + 32],
                                        in0=zm_ps[h2][:],
                                        in1=m2[:, co:co + 32], op=ALU.mult)

            # ---- Hopfield #2 -> ship q2/64 (cols 0:64) and s2 (col 64);
            # the host computes out = -q~2/s2' with s2' = -s2/64.
            ST2 = [ps.tile([128, 4, 32], F32, tag="S", bufs=2, name="ST2a"),
                   ps.tile([128, 4, 32], F32, tag="S", bufs=2, name="ST2b")]
            for h2 in range(2):
                co = 32 * h2
                for t in range(4):
                    nc.tensor.matmul(ST2[h2][:, t, :],
                                     KT[:, t * 128:(t + 1) * 128],
                                     z2m[:, co:co + 32],
                                     start=True, stop=True)
            att2 = sb.tile([128, 4, 64], F16, tag="att2", name="att2")
            for h2 in range(2):
                co = 32 * h2
                nc.scalar.activation(out=att2[:, :, co:co + 32],
                                     in_=ST2[h2][:], func=AF.Exp,
                                     bias=0.0, scale=BETA)
            qs2 = ps.tile([64, 65], F32, tag="q64", bufs=2, name="qs2")
            for h2 in range(2):
                co = 32 * h2
                for t in range(4):
                    nc.tensor.matmul(qs2[:, co:co + 32], KV[:, t, :],
                                     att2[:, t, co:co + 32],
                                     start=(t == 0), stop=(t == 3))
            for t in range(4):
                nc.tensor.matmul(qs2[:, 64:65], att2[:, t, :],
                                 neg_col[:], start=(t == 0), stop=(t == 3))
            out_sb = sb.tile([64, 65], F16, tag="out_sb")
            nc.vector.tensor_copy(out=out_sb[:], in_=qs2[:])
            nc.sync.dma_start(out=out_t[:], in_=out_sb[:])
    nc.compile()
    return nc


def _get_nc(debug=False):
    key = ("nc", debug)
    if key not in _CACHE:
        _CACHE[key] = _build_nc(debug)
    return _CACHE[key]


# ---------------------------------------------------------------- entry point
def kernel(x, w1, b1, w2, b2, K, Vw, _debug=False):
    x = np.asarray(x, np.float32)
    shared = _host_prep(np.asarray(w1, np.float32), np.asarray(b1, np.float32),
                        np.asarray(w2, np.float32), np.asarray(b2, np.float32),
                        np.asarray(K, np.float32), np.asarray(Vw, np.float32))
    bsz = x.shape[0]
    nc = _get_nc(False)
    smpls = [_sample_prep(x[b]) for b in range(bsz)]
    in_maps = []
    for core in range(N_CORES):
        P1b, Xb = smpls[core] if core < bsz else smpls[0]
        mainb = shared["main"].copy()
        mainb[0:49, 0:256] = P1b
        m = {"main": mainb, "cv2": shared["cv2"], "hop": shared["hop"],
             "wB": shared["wB"], "wC": shared["wC"], "smpl": Xb}
        in_maps.append(m)
    res = run_bass_kernel_spmd(nc, in_maps, core_ids=list(range(N_CORES)))
    outs = []
    for b in range(bsz):
        r = np.asarray(res.results[b]["out"], np.float32)
        # col 64 holds -s2/64, so q2_true = -q~2 / (-s2/64)
        outs.append((-r[:, 0:64] / r[:, 64:65].T).reshape(64, 8, 8))
    out = np.stack(outs).astype(np.float32)
    if _debug:
        return out, res
    return out
